# revision 1
# baseline (speedup 1.0000x reference)
"""HDClassifier Trainium2 kernel, v2.

Math (per batch b):
  idx[t,c]   = clip(round((x+100)/200*200), 0, 200)
  bundled[t] = sum_c level_hv[idx[t,c]] * channel_hv[c]   # even ints in [-8,8]
  u[t,j]     = bundled[t,j] * bundled[t+1,j+1]            # 4*a*b, |.|<=64, fp8e4m3-exact
  gram[t',m] = u[t',m] * u[t'+2,m+2]                      # mult of 16, |.|<=4096, bf16-exact
  sample[m]  = sum_t' gram[t',m]
  out        = sign(sample) @ centroid.T

Sharding: 8 cores = 2 batch-groups x 4 d-slices. Core (gb, gd) handles
batches [16*gb, 16*gb+16) and d-window [2500*gd - 3, 2500*gd + 2500) with the
3-col halo handled by a host-side circular rotation of the table slice, so
the device sees a plain [0, 2503) column space and needs no wrap DMAs.

Per-core engine budget (cost model): PE ~71us (one-hot DR matmuls + part of
the t'-reduce), DVE/Pool share the elementwise muls and the rest of the
reduce (gpsimd partition_all_reduce), ACT drains PSUM, DMA ~20MB.
"""

import sys

sys.path.insert(0, "/opt/trn_rl_repo")

import numpy as np
from collections import deque

import concourse.bass as bass
import concourse.bass_isa as bass_isa
import concourse.mybir as mybir
from concourse import bacc
from concourse.bass_utils import run_bass_kernel_spmd
from concourse.tile import TileContext

# Problem constants
NUM_LEVELS = 201
N_GRAM = 4
B, T, C, D, NUM_CLASSES = 32, 128, 8, 10000, 6
N_CORES = 8
GB, GD = 2, 4              # batch groups x d-slices
B_LOC = B // GB            # 16 batches per core
W = D // GD                # 2500 output cols per core
W3 = W + 3                 # 2503 cols incl. left halo
K_TOT = C * NUM_LEVELS     # 1608
KP = 7                     # DoubleRow k-passes of 256
K_PAD = KP * 256           # 1792
NTU = T - 1                # 127 u rows
NTP = T - N_GRAM + 1       # 125 gram rows

# Phase-A / reduce chunking (PSUM bank = 512 f32)
A_CHUNKS = [(0, 512), (512, 512), (1024, 512), (1536, 512), (2048, 455)]
R_CHUNKS = [(0, 512), (512, 512), (1024, 512), (1536, 512), (2048, 452)]

# Engine assignment (batch index -> engine), tuned against the cost model.
POOL_U = frozenset()               # u-muls all on DVE (bf16 2x mode)
POOL_G = frozenset()               # gram-muls all on DVE
PE_R = [12, 13, 14, 15]            # last batches' t'-reduce on PE (at the end)
REDUCE_AT_END = 1
RED_LAG = 3
BF16_STAGE = 1                     # bf16 staging -> DVE 2x muls
FP8_B = frozenset()                # no per-batch fp8 staging
POOL_R = frozenset(range(B_LOC)) - set(PE_R)       # the rest reduce on Pool
N_PE_R = len(PE_R)
STAGE_DT = None  # set below once dtypes are defined

FP8 = mybir.dt.float8e4
BF16 = mybir.dt.bfloat16
F32 = mybir.dt.float32
STAGE_DT = BF16 if BF16_STAGE else FP8
NP_FP8 = np.dtype(mybir.dt.np(FP8))
NP_BF16 = np.dtype(mybir.dt.np(BF16))

_CACHE = {}


def _build_program():
    nc = bacc.Bacc("TRN2", target_bir_lowering=False, debug=False, num_devices=N_CORES)

    table_p = nc.declare_dram_parameter("table", [128, KP, 2, W3], FP8, isOutput=False)
    oh_p = nc.declare_dram_parameter("onehot", [128, B_LOC, KP, 2, T], FP8, isOutput=False)
    sel_p = nc.declare_dram_parameter("sel", [128, B_LOC * B_LOC], BF16, isOutput=False)
    out_p = nc.declare_dram_parameter("sample", [B_LOC, W], F32, isOutput=True)

    with TileContext(nc) as tc:
        with (
            tc.tile_pool(name="const", bufs=1) as cpool,
            tc.tile_pool(name="lb", bufs=5) as lbpool,
            tc.tile_pool(name="sh", bufs=3) as shpool,
            tc.tile_pool(name="u", bufs=3) as upool,
            tc.tile_pool(name="ush", bufs=3) as ushpool,
            tc.tile_pool(name="gram", bufs=(N_PE_R + 2 if REDUCE_AT_END else 5)) as gpool,
            tc.tile_pool(name="pall", bufs=2) as papool,
            tc.tile_pool(name="samp", bufs=1) as spool,
            tc.tile_pool(name="psA", bufs=3, space="PSUM") as psA_pool,
            tc.tile_pool(name="psB", bufs=5, space="PSUM") as psB_pool,
        ):
            # Early input loads on the ACT HWDGE queue, ordered to feed the
            # chunk-major phase-A warm-up (the DMA bus is FIFO by issue time).
            # The late onehot block is Pool-gated further below.
            sel_sb = cpool.tile([128, B_LOC * B_LOC], BF16, tag="sel")
            table_sb = cpool.tile([128, KP, 2, W3], FP8, tag="table")
            oh_sb = cpool.tile([128, B_LOC, KP, 2, T], FP8, tag="oh")
            # Bus order: oh[0:2] (warm-up batches), then the whole table (the
            # warm-up is table-stream-bound), then oh[2:6].
            # Table in 4 DMAs; the last covers cols [1536, 2503) so every
            # descriptor run is >= 512B (smaller runs pay a 2x bus penalty).
            TAB_LOADS = [(0, 512), (512, 512), (1024, 512), (1536, 967)]
            # Zero the k-pad regions (kp6/j1 fully, kp6/j0 partitions 72..):
            # the one-hot lhsT is zero there, but uninitialized SBUF bytes can
            # be fp8-NaN and NaN*0 = NaN would poison the psum. Memsets are on
            # Pool (idle early) and the loads below skip those 0.46MB.
            nc.gpsimd.memset(table_sb[:, 6, 1, :], 0)
            nc.gpsimd.memset(table_sb[64:128, 6, 0, :], 0)
            nc.scalar.dma_start(out=oh_sb[:, 0:2], in_=oh_p[:, 0:2])
            for c0, cw in TAB_LOADS:
                nc.scalar.dma_start(
                    out=table_sb[:, 0:6, :, c0 : c0 + cw],
                    in_=table_p[:, 0:6, :, c0 : c0 + cw],
                )
                nc.scalar.dma_start(
                    out=table_sb[0:72, 6, 0, c0 : c0 + cw],
                    in_=table_p[0:72, 6, 0, c0 : c0 + cw],
                )
            nc.scalar.dma_start(out=oh_sb[:, 2:4], in_=oh_p[:, 2:4])
            nc.scalar.dma_start(out=oh_sb[:, 4:6], in_=oh_p[:, 4:6])
            nc.scalar.dma_start(out=sel_sb[:], in_=sel_p[:])

            # The late onehot block is gated behind the third table piece
            # via a tiny "reservation" write into its target region (a real
            # WAW dep), placing its 2.6MB mid-stream on the FIFO DMA bus.
            # The scheduler issues no-dep DMAs immediately, so without the
            # gate these bytes would occupy the bus ahead of the first
            # phase-B staging transfers.
            nc.gpsimd.tensor_copy(
                out=oh_sb[0:1, 6, 0, 0, 0:2], in_=table_sb[0:1, 0, 0, 1024:1026]
            )
            nc.gpsimd.dma_start(out=oh_sb[:, 6:16], in_=oh_p[:, 6:16])

            dmy = cpool.tile([128, 512], BF16, tag="dmy")
            nc.gpsimd.memset(dmy[:], 0)
            psB = [
                psB_pool.tile([max(N_PE_R, 1), cw], F32, tag="psB", name=f"psB{i}")
                for i, (c0, cw) in enumerate(R_CHUNKS)
            ] if N_PE_R else []
            samp = spool.tile([max(N_PE_R, 1), W], F32, tag="samp", name="samp") if N_PE_R else None

            grams = {}

            palls = {}

            def reduce_pool(b):
                """t'-reduce of gram[b] on Pool; row DMA deferred (see ship_row)."""
                gram = grams.pop(b)
                pall = papool.tile([NTP, W], F32, tag="pall")
                nc.gpsimd.partition_all_reduce(
                    pall[:], gram[:NTP, :], channels=NTP, reduce_op=bass_isa.ReduceOp.add
                )
                palls[b] = pall

            def ship_row(b):
                """DMA row 0 of pall[b] (the t'-sum) to DRAM row b. Emitted a
                couple of steps after the pall op so its sem wait is already
                satisfied and doesn't head-of-line-block the SP queue."""
                pall = palls.pop(b)
                nc.sync.dma_start(out=out_p[b : b + 1, :], in_=pall[0:1, :])

            def reduce_pe(b, ext_start=False):
                """t'-reduce of gram[b] via selection matmul -> psB row."""
                gram = grams.pop(b)
                r = PE_R.index(b)
                lhsT = sel_sb[:NTP, r * N_PE_R : (r + 1) * N_PE_R]
                for i, (c0, cw) in enumerate(R_CHUNKS):
                    nc.tensor.matmul(
                        psB[i][:],
                        lhsT,
                        gram[:NTP, c0 : c0 + cw],
                        start=(b == PE_R[0] and not ext_start),
                        stop=(b == PE_R[-1]),
                    )

            def dummy_set(first, gate):
                """Zero-contribution matmuls into psB: keep the PE busy (and
                its p-state up) while waiting for the tail grams. lhsT is an
                all-zero block of sel, so psB is unchanged. `gate` is a
                late-pipeline tile used as rhs purely so the scheduler can't
                hoist these to the front of the PE queue."""
                z = sel_sb[:NTP, 32 : 32 + N_PE_R]
                for i, (c0, cw) in enumerate(R_CHUNKS):
                    nc.tensor.matmul(
                        psB[i][:], z, gate[:NTP, 0:cw], start=first, stop=False,
                        skip_group_check=not first,
                    )

            us = {}
            ushes = {}

            def phase_b1(b):
                """bundled[b] -> u (sh1 staging DMA + mul)."""
                dt = FP8 if b in FP8_B else STAGE_DT
                lb = lbs.pop(b)
                sh1 = shpool.tile([NTU, W + 2], dt, tag="sh1")
                nc.sync.dma_start(out=sh1[:], in_=lb[1:T, 1 : W3])
                u = upool.tile([NTU, W + 2], dt, tag="u")
                eng_u = nc.gpsimd if b in POOL_U else nc.vector
                eng_u.tensor_mul(out=u[:], in0=lb[:NTU, 0 : W + 2], in1=sh1[:])
                us[b] = u

            def phase_b2(b):
                """u -> gram (ush staging DMA + mul), one step later so the
                ush DMA's sem wait doesn't hold the SP sequencer."""
                u = us.pop(b)
                dt = FP8 if b in FP8_B else STAGE_DT
                ush = ushpool.tile([NTP, W], dt, tag="ush")
                ushes[b] = ush
                nc.sync.dma_start(out=ush[:], in_=u[2 : NTP + 2, 2 : W + 2])
                gram = gpool.tile([128, W], BF16, tag="gram")
                eng_g = nc.gpsimd if b in POOL_G else nc.vector
                eng_g.tensor_mul(out=gram[:NTP, :], in0=u[:NTP, 0:W], in1=ush[:])
                grams[b] = gram

            lbs = {}
            psa_ctr = [0]

            def a_chunk(b, ci):
                """Phase A for (batch b, chunk ci): 7 DR matmuls + drain."""
                c0, cw = A_CHUNKS[ci]
                ps = psA_pool.tile(
                    [128, 512], F32, tag="psA", name=f"psA{psa_ctr[0] % 3}"
                )
                psa_ctr[0] += 1
                for kp in range(KP):
                    nc.tensor.matmul(
                        ps[:, :cw],
                        oh_sb[:, b, kp, :, :],
                        table_sb[:, kp, :, c0 : c0 + cw],
                        start=(kp == 0),
                        stop=(kp == KP - 1),
                        perf_mode=mybir.MatmulPerfMode.DoubleRow,
                    )
                nc.scalar.copy(out=lbs[b][:, c0 : c0 + cw], in_=ps[:, :cw])

            # Warm-up: chunk-major over the first NB0 batches so PE work
            # overlaps the table-chunk stream (the first batch alone would be
            # table-bandwidth-bound).
            NB0 = 2
            for b in range(NB0):
                lbs[b] = lbpool.tile([128, W3], FP8 if b in FP8_B else STAGE_DT, tag="lb", name=f"lb{b}")
            for ci in range(len(A_CHUNKS)):
                for b in range(NB0):
                    a_chunk(b, ci)

            # Pipelined emission: one stage-advance per step per queue.
            q1 = deque((0, b) for b in range(NB0))
            q2, qr, qs = deque(), deque(), deque()
            grams_hold = []
            for s in range(NB0, B_LOC + 12):
                # Once phase-A emission is done the queues can drain several
                # stages per step (no PE work left to interleave with).
                k = 1 if s < B_LOC else 3
                if s < B_LOC:
                    b = s
                    lbs[b] = lbpool.tile([128, W3], FP8 if b in FP8_B else STAGE_DT, tag="lb", name=f"lb{b}")
                    for ci in range(len(A_CHUNKS)):
                        a_chunk(b, ci)
                    q1.append((s, b))
                for _ in range(k):
                    if q1 and q1[0][0] < s:
                        b = q1.popleft()[1]
                        phase_b1(b)
                        q2.append((s, b))
                for _ in range(k):
                    if q2 and q2[0][0] < s:
                        b = q2.popleft()[1]
                        phase_b2(b)
                        qr.append((s, b))
                for _ in range(k):
                    # Pool reduces chase the gram stream closely; PE reduces
                    # lag RED_LAG extra steps so the in-order PE queue never
                    # reaches one before its gram exists (a stalled matmul
                    # blocks phase A behind it and drops the PE p-state).
                    if qr and qr[0][1] in POOL_R and qr[0][0] < s:
                        b = qr.popleft()[1]
                        reduce_pool(b)
                        qs.append((s, b))
                    elif qr and qr[0][1] not in POOL_R and qr[0][0] < s - RED_LAG:
                        b = qr.popleft()[1]
                        if not REDUCE_AT_END:
                            reduce_pe(b)
                        else:
                            grams_hold.append(b)
                if qs and qs[0][0] <= s - 2:
                    ship_row(qs.popleft()[1])

            for _, b in qs:
                ship_row(b)
            if REDUCE_AT_END:
                N_PRE, N_MID = 0, 0
                gates = [ushes[b] for b in sorted(ushes)[-7:]]
                gi = [0]

                def next_gate():
                    g = gates[min(gi[0] // 3, len(gates) - 1)]
                    gi[0] += 1
                    return g

                for i in range(N_PRE):
                    dummy_set(first=(i == 0), gate=next_gate())
                for j, b in enumerate(grams_hold):
                    if j:
                        for _ in range(N_MID):
                            dummy_set(first=False, gate=next_gate())
                    reduce_pe(b, ext_start=(N_PRE > 0))

            # Drain psB chunks into samp (row r = batch PE_R[r], a contiguous
            # block) in parallel across ACT/DVE, shipping each chunk to DRAM
            # as soon as it lands.
            def drain_act(o, i_):
                nc.scalar.copy(out=o, in_=i_)

            def drain_dve(o, i_):
                nc.vector.tensor_copy(out=o, in_=i_)

            drain_eng = [drain_act, drain_dve, drain_act, drain_dve, drain_act]
            for i, (c0, cw) in enumerate(R_CHUNKS):
                drain_eng[i](samp[:, c0 : c0 + cw], psB[i][:])
            nc.sync.dma_start(out=out_p[PE_R[0] : PE_R[-1] + 1, :], in_=samp[:])

    nc.finalize()
    return nc


def _host_prep(x, level_hv, channel_hv):
    # Bit-exact replication of the jax fp32 quantization
    x = np.asarray(x, dtype=np.float32)
    t1 = x + np.float32(100.0)
    t2 = t1 / np.float32(200.0)
    t3 = t2 * np.float32(200.0)
    idx = np.clip(np.rint(t3), 0, NUM_LEVELS - 1).astype(np.int32)  # [B,T,C]

    fp8_one = np.array([1.0], dtype=np.float32).astype(NP_FP8)[0]
    fp8_mone = np.array([-1.0], dtype=np.float32).astype(NP_FP8)[0]

    # folded +-1 table, padded to 1792 rows
    prod = (level_hv[None, :, :] * channel_hv[:, None, :]).reshape(K_TOT, D)
    tab = np.zeros((K_PAD, D), dtype=NP_FP8)
    tab[:K_TOT] = np.where(prod > 0, fp8_one, fp8_mone)

    # per-d-slice rotated table: [128, KP, 2, W3]
    tabs = []
    for gd in range(GD):
        cols = (gd * W - 3 + np.arange(W3)) % D
        tr = np.ascontiguousarray(
            tab[:, cols].reshape(KP, 2, 128, W3).transpose(2, 0, 1, 3)
        )
        tabs.append(tr)

    # one-hot planes: [B, K_PAD, T]
    oh = np.zeros((B, K_PAD, T), dtype=NP_FP8)
    bb, tt, cc = np.meshgrid(np.arange(B), np.arange(T), np.arange(C), indexing="ij")
    kk = cc * NUM_LEVELS + idx
    oh[bb.ravel(), kk.ravel(), tt.ravel()] = fp8_one
    # per batch-group: [128, B_LOC, KP, 2, T]
    ohs = []
    for gb in range(GB):
        og = np.ascontiguousarray(
            oh[gb * B_LOC : (gb + 1) * B_LOC]
            .reshape(B_LOC, KP, 2, 128, T)
            .transpose(3, 0, 1, 2, 4)
        )
        ohs.append(og)

    # selection columns: block r (for the r-th PE-reduced batch) is the
    # one-hot column r over rows k < NTP
    sel = np.zeros((128, B_LOC * B_LOC), dtype=np.float32)
    for r in range(N_PE_R):
        sel[:NTP, r * N_PE_R + r] = 1.0
    sel = sel.astype(NP_BF16)
    return tabs, ohs, sel


def kernel(x, level_hv, channel_hv, centroid):
    if "nc" not in _CACHE:
        _CACHE["nc"] = _build_program()
    nc = _CACHE["nc"]

    tabs, ohs, sel = _host_prep(x, level_hv, channel_hv)

    in_maps = []
    for core in range(N_CORES):
        gb, gd = divmod(core, GD)
        in_maps.append({"table": tabs[gd], "onehot": ohs[gb], "sel": sel})

    res = run_bass_kernel_spmd(nc, in_maps, list(range(N_CORES)))
    _CACHE["last_results"] = res

    sample = np.zeros((B, D), dtype=np.float32)
    for core in range(N_CORES):
        gb, gd = divmod(core, GD)
        sample[gb * B_LOC : (gb + 1) * B_LOC, gd * W : (gd + 1) * W] = res.results[
            core
        ]["sample"].reshape(B_LOC, W)
    sign = np.where(sample > 0, np.float32(1.0), np.float32(-1.0))
    return (sign @ np.asarray(centroid, dtype=np.float32).T).astype(np.float32)



# revision 2
# speedup vs baseline: 1.1902x; 1.1902x over previous
"""HDClassifier Trainium2 kernel, v3 — j-interleaved layout.

Math (per batch b):
  idx[t,c]   = clip(round((x+100)/200*200), 0, 200)
  bundled[t,d] = sum_c level_hv[idx[t,c],d] * channel_hv[c,d]   # even ints in [-8,8]
  gram[t',d] = prod_i bundled[t'+i, d-3+i]                      # |.|<=4096, bf16-exact
  sample[d]  = sum_t' gram[t',d];  out = sign(sample) @ centroid.T

Sharding: 8 cores = 2 batch-groups x 4 d-slices (GB x GD). Core (gb, gd)
handles batches [16*gb, 16*gb+16) and d-window [2500*gd - 3, +2503) (left
halo of 3 via host-side circular rotation).

Layout trick: local column dl = NJ*p + j (NJ=20, p = SBUF partition,
j in [0,20)). Phase A computes b_j[p, (b,t)] = bundled[t, NJ*p+j] via
matmuls with lhsT = table j-slice (fp8 DoubleRow, k=1792 in 7 passes),
rhs = one-hot columns (b,t). In this layout ALL the +1/+2 shifts in t and
d are free-dim AP offsets (d+1 == j+1 -> next b-tile, t+1 == col+1), so
the 20MB of SBUF staging DMAs in v2 disappear:
  u_j    = b_j * b_{j+1}(col+1)          (DVE bf16 2x)
  gram_j = u_j * u_{j+2}(col+2)          (DVE bf16 2x, packed (b,t') out)
j = 20..22 wrap to partition p+1 and are made by 3 small SBUF halo DMAs.
The t'-reduce has no cheap on-device home in this layout (free-dim reduce
is DVE-only at 1x ~43us); gram is bf16-EXACT (multiples of 16, |.|<=4096)
so it ships to DRAM (10MB, under the PE roofline) and the host does the
final exact sum + sign + centroid dot.

Cost model budget per core: PE 59.7us (7 k-passes x 40960 cols x 0.5 x
0.42ns) = bottleneck; DMA 55us (8.3 in + 1.6 halo + 10.0 gram out);
DVE ~50us (u+gram muls); ACT ~46us (psum drains); Pool ~0 (issues DMAs).
Columns are processed in two 1024-col halves so the first drain only
waits for half the one-hot stream.
"""

import sys

sys.path.insert(0, "/opt/trn_rl_repo")

import numpy as np

import concourse.bass as bass
import concourse.mybir as mybir
from concourse import bacc
from concourse.bass_utils import run_bass_kernel_spmd
from concourse.tile import TileContext

# Problem constants
NUM_LEVELS = 201
B, T, C, D = 32, 128, 8, 10000
N_CORES = 8
GB, GD = 2, 4
B_LOC = B // GB            # 16 batches per core
W = D // GD                # 2500 output cols per core
W3 = W + 3                 # incl. left halo
NJ = 20                    # d interleave: dl = NJ*p + j
NP = 128                   # partitions used by phase A
KP = 7                     # DoubleRow k-passes of 256
K_PAD = KP * 256           # 1792
K_TOT = C * NUM_LEVELS     # 1608
NH = 2                     # column halves
BH = B_LOC // NH           # 8 batches per half
COLS = BH * T              # 1024
NTP = T - 3                # 125 valid t' per batch
SHIP_P = 125               # partitions shipped (dl = NJ*p + j < 2500)

FP8 = mybir.dt.float8e4
BF16 = mybir.dt.bfloat16
F32 = mybir.dt.float32
NP_FP8 = np.dtype(mybir.dt.np(FP8))
NP_BF16 = np.dtype(mybir.dt.np(BF16))

_CACHE = {}


def _build_program():
    nc = bacc.Bacc("TRN2", target_bir_lowering=False, debug=False, num_devices=N_CORES)

    tab_p = nc.declare_dram_parameter("tab", [128, NJ, KP, 2, NP], FP8, isOutput=False)
    oh_p = nc.declare_dram_parameter("oh", [NH, 128, KP, 2, COLS], FP8, isOutput=False)
    gr_p = nc.declare_dram_parameter("gr", [NH, NJ, SHIP_P, BH * NTP], BF16, isOutput=True)
    # Ship groups of j's per DMA to amortize the ~1us issue cost; groups
    # follow the production order of JORDER below, and the tail groups are
    # small so the last ship chases the last gram closely.
    SHIP_GROUPS = [(17, 20), (0, 4), (4, 8), (8, 12), (12, 14), (14, 16), (16, 17)]
    GRP_OF = {}
    for gi, (j0, j1) in enumerate(SHIP_GROUPS):
        for j in range(j0, j1):
            GRP_OF[j] = gi

    with TileContext(nc) as tc:
        with (
            tc.tile_pool(name="const", bufs=1) as cpool,
            tc.tile_pool(name="b", bufs=6) as bpool,
            tc.tile_pool(name="u", bufs=8) as upool,
            tc.tile_pool(name="gram", bufs=3) as gpool,
            tc.tile_pool(name="ps", bufs=4, space="PSUM") as pspool,
        ):
            tab = cpool.tile([128, NJ, KP, 2, NP], FP8, tag="tab")
            ohs = [cpool.tile([128, KP, 2, COLS], FP8, tag=f"oh{h}", name=f"oh{h}") for h in range(NH)]

            # Input loads. One-hot kp-blocks go on the ACT HWDGE queue while
            # table j-blocks go in 4-j batches on the SP HWDGE queue — the
            # two descriptor generators run in parallel, so the head matmuls
            # (table j0-3 + oh kp0) are ready ~4us in. Ships use the Pool
            # SWDGE queue (see gram_ship), keeping SP/ACT free mid-flight.
            def load_oh(h, kp, eng):
                eng.dma_start(out=ohs[h][:, kp], in_=oh_p[h, :, kp])

            nc.sync.dma_start(out=tab[:, 0:2], in_=tab_p[:, 0:2])
            load_oh(0, 0, nc.scalar)
            nc.sync.dma_start(out=tab[:, 2:4], in_=tab_p[:, 2:4])
            for kp in range(1, KP):
                load_oh(0, kp, nc.scalar)
            # Gate the remaining loads so the half-0 one-hot blocks keep bus
            # priority (bus is FIFO by descriptor-gen completion): a tiny
            # copy reads an already-requested one-hot region (RAW dep on its
            # DMA) and writes into the gated table region (WAW dep for the
            # DMA below). SP-queue DMAs stay ordered among themselves.
            nc.gpsimd.tensor_copy(out=tab[0:1, 17, 0, 0, 0:2], in_=ohs[0][0:1, 2, 0, 0:2])
            nc.sync.dma_start(out=tab[:, 17:20], in_=tab_p[:, 17:20])
            nc.gpsimd.tensor_copy(out=tab[0:1, 4, 0, 0, 0:2], in_=ohs[0][0:1, 5, 0, 0:2])
            for j0, j1 in ((4, 8), (8, 13), (13, 17)):
                nc.sync.dma_start(out=tab[:, j0:j1], in_=tab_p[:, j0:j1])
            for kp in range(KP):
                load_oh(1, kp, nc.sync)

            # Per-half pipelines. j order: 0-3 first (halos + head), then
            # 17-19 so the high-j u's (which the +2-chained grams need)
            # exist early; the tail after the last drain (b_16) is just
            # u_15, u_16 and grams 13..16.
            JORDER = [0, 1, 2, 3, 17, 18, 19] + list(range(4, 17))

            for h in range(NH):
                oh = ohs[h]
                bt = {}      # j -> b tile (j in 0..19), 20..22 are halos
                ut = {}      # j -> u tile

                def mm_j(j, ps):
                    for g in range(2):
                        for kp in range(KP):
                            nc.tensor.matmul(
                                ps[:, g * 512:(g + 1) * 512],
                                tab[:, j, kp],
                                oh[:, kp, :, g * 512:(g + 1) * 512],
                                start=(kp == 0),
                                stop=(kp == KP - 1),
                                perf_mode=mybir.MatmulPerfMode.DoubleRow,
                            )

                def drain_j(j, ps):
                    b = bpool.tile([128, COLS + 8], BF16, tag="b", name=f"b{h}_{j}")
                    nc.gpsimd.memset(b[:, COLS:], 0)
                    nc.scalar.copy(out=b[:, 0:COLS], in_=ps[:])
                    bt[j] = b
                    if j < 3:
                        # halo: b_{20+j}[p] = b_j[p+1] (partition-shift DMA)
                        hl = cpool.tile([127, COLS + 2], BF16, tag=f"halo{h}_{j}", name=f"halo{h}_{j}")
                        nc.gpsimd.dma_start(out=hl[:], in_=b[1:128, 0:COLS + 2])
                        bt[NJ + j] = hl

                def try_u(j):
                    # u_j = b_j * b_{j+1}(col+1); j in [0, 22)
                    if j in ut or j not in bt or j + 1 not in bt:
                        return False
                    b0 = bt[j]
                    b1 = bt[j + 1]
                    np_ = 128 if j + 1 < NJ else 127
                    u = upool.tile([128, COLS], BF16, tag="u", name=f"u{h}_{j}")
                    nc.vector.tensor_mul(
                        out=u[0:np_, :],
                        in0=b0[0:np_, 0:COLS],
                        in1=b1[0:np_, 1:COLS + 1],
                    )
                    ut[j] = (u, np_)
                    return True

                gtiles = {}

                def gram_ship(j):
                    # gram_j = u_j * u_{j+2}(col+2), packed (b, t') into a
                    # group tile; ship the group when its last j lands.
                    gi = GRP_OF[j]
                    j0, j1 = SHIP_GROUPS[gi]
                    if j == j0:
                        gtiles[gi] = gpool.tile(
                            [SHIP_P, (j1 - j0) * BH * NTP], BF16,
                            tag=f"gram{j1 - j0}", name=f"g{h}_{gi}"
                        )
                    g = gtiles[gi]
                    jj = j - j0
                    u0, np0 = ut[j]
                    u2, np2 = ut[j + 2]
                    in0 = u0[0:SHIP_P, 0:COLS].rearrange("p (b t) -> p b t", b=BH)[:, :, 0:NTP]
                    in1 = u2[0:SHIP_P, 0:COLS].rearrange("p (b t) -> p b t", b=BH)[:, :, 2:NTP + 2]
                    out = g[:, jj * BH * NTP:(jj + 1) * BH * NTP].rearrange(
                        "p (b t) -> p b t", b=BH
                    )
                    nc.vector.tensor_mul(out=out, in0=in0, in1=in1)
                    if j == j1 - 1:
                        nc.gpsimd.dma_start(
                            out=gr_p[h, j0:j1].rearrange("j p c -> p j c"),
                            in_=g[:].rearrange("p (j c) -> p j c", j=j1 - j0),
                        )

                gdone = set()

                def emit_grams():
                    for j in range(NJ):
                        if j not in gdone and j in ut and j + 2 in ut:
                            gram_ship(j)
                            gdone.add(j)

                def advance():
                    # Grams whose u's already exist go on the DVE queue FIRST
                    # (their deps are old), then the new u's, then grams those
                    # unlock — avoids head-of-line stalls on the in-order
                    # DVE queue.
                    emit_grams()
                    for j in range(NJ + 2):
                        try_u(j)
                    emit_grams()

                # Head: kp-major over j0-1, then j2-3, so PE starts on the
                # first one-hot kp-block and the first drains (which pace
                # the whole DVE pipeline) land as early as possible.
                HEAD = JORDER[:4]
                head_ps = {j: pspool.tile([128, COLS], F32, tag="ps", name=f"psh{h}_{j}") for j in HEAD}
                for jpair in (HEAD[0:2], HEAD[2:4]):
                    for kp in range(KP):
                        for j in jpair:
                            for g in range(2):
                                nc.tensor.matmul(
                                    head_ps[j][:, g * 512:(g + 1) * 512],
                                    tab[:, j, kp],
                                    oh[:, kp, :, g * 512:(g + 1) * 512],
                                    start=(kp == 0),
                                    stop=(kp == KP - 1),
                                    perf_mode=mybir.MatmulPerfMode.DoubleRow,
                                )
                    for j in jpair:
                        drain_j(j, head_ps[j])
                        advance()

                for j in JORDER[4:]:
                    ps = pspool.tile([128, COLS], F32, tag="ps", name=f"ps{h}_{j}")
                    mm_j(j, ps)
                    drain_j(j, ps)
                    advance()
                advance()
                assert len(gdone) == NJ, f"half {h}: grams stuck at {sorted(set(range(NJ)) - gdone)}"

    nc.finalize()
    return nc


def _host_prep(x, level_hv, channel_hv):
    # Bit-exact replication of the jax fp32 quantization
    x = np.asarray(x, dtype=np.float32)
    t1 = x + np.float32(100.0)
    t2 = t1 / np.float32(200.0)
    t3 = t2 * np.float32(200.0)
    idx = np.clip(np.rint(t3), 0, NUM_LEVELS - 1).astype(np.int32)  # [B,T,C]

    fp8_one = np.float32(1.0).astype(NP_FP8)
    fp8_mone = np.float32(-1.0).astype(NP_FP8)

    # folded +-1 table [K_TOT, D] -> fp8, k = c*201 + level
    prod = (channel_hv[:, None, :] * level_hv[None, :, :]).reshape(K_TOT, D)
    tabf = np.where(prod > 0, fp8_one, fp8_mone)

    # per-core tables: [128, NJ, KP, 2, NP], lhsT[k, m=p] = tab[k, dl=NJ*p+j]
    # dl -> global: (gd*W - 3 + dl) mod D; dl >= W3 columns are zero.
    tabs = []
    for gd in range(GD):
        dls = NJ * np.arange(NP)[None, :] + np.arange(NJ)[:, None]  # [NJ, NP]
        cols = (gd * W - 3 + dls) % D
        tcore = np.zeros((K_PAD, NJ, NP), dtype=NP_FP8)
        valid = dls < W3
        tcore[:K_TOT][:, valid] = tabf[:, cols[valid]]
        # k = kp*256 + r*128 + part -> [part, NJ, kp, r, NP]
        tabs.append(np.ascontiguousarray(
            tcore.reshape(KP, 2, 128, NJ, NP).transpose(2, 3, 0, 1, 4)))

    # one-hots: [NH, 128, KP, 2, COLS] per batch-group
    ohs = []
    for gb in range(GB):
        oh = np.zeros((K_PAD, B_LOC * T), dtype=NP_FP8)
        sl = idx[gb * B_LOC:(gb + 1) * B_LOC]          # [B_LOC, T, C]
        bb, tt, cc = np.meshgrid(np.arange(B_LOC), np.arange(T), np.arange(C), indexing="ij")
        kk = (cc * NUM_LEVELS + sl).ravel()
        oh[kk, (bb * T + tt).ravel()] = fp8_one
        o = oh.reshape(KP, 2, 128, NH, COLS).transpose(3, 2, 0, 1, 4)
        ohs.append(np.ascontiguousarray(o))
    return tabs, ohs


def kernel(x, level_hv, channel_hv, centroid):
    if "nc" not in _CACHE:
        _CACHE["nc"] = _build_program()
    nc = _CACHE["nc"]

    tabs, ohs = _host_prep(x, level_hv, channel_hv)

    in_maps = []
    for core in range(N_CORES):
        gb, gd = divmod(core, GD)
        in_maps.append({"tab": tabs[gd], "oh": ohs[gb]})

    res = run_bass_kernel_spmd(nc, in_maps, list(range(N_CORES)))
    _CACHE["last_results"] = res

    sample = np.zeros((B, D), dtype=np.float32)
    for core in range(N_CORES):
        gb, gd = divmod(core, GD)
        gr = res.results[core]["gr"]  # [NH, NJ, SHIP_P, BH*NTP] bf16
        g32 = (np.ascontiguousarray(gr).view(np.uint16).astype(np.uint32) << 16).view(np.float32)
        s = g32.reshape(NH, NJ, SHIP_P, BH, NTP).sum(axis=-1)  # [NH, NJ, P, BH]
        # sample[b_global, d_global]: b = gb*16 + h*BH + b', d = gd*W + NJ*p + j
        dl = (NJ * np.arange(SHIP_P)[None, :] + np.arange(NJ)[:, None])  # [NJ, P]
        ok = dl < W
        for hh in range(NH):
            for b2 in range(BH):
                row = np.zeros(W, dtype=np.float32)
                row[dl[ok]] = s[hh, :, :, b2][ok]
                sample[gb * B_LOC + hh * BH + b2, gd * W:(gd + 1) * W] = row
    sign = np.where(sample > 0, np.float32(1.0), np.float32(-1.0))
    return (sign @ np.asarray(centroid, dtype=np.float32).T).astype(np.float32)


# revision 3
# speedup vs baseline: 1.2246x; 1.0289x over previous
"""HDClassifier Trainium2 kernel, v4 — j-interleaved layout, 6 k-passes.

Same architecture as v3 (see its docstring) but sharded GB=4 x GD=2:
each core handles 8 batches x 5000 d-columns. With only 8 batches per
group, the number of DISTINCT (channel, level) pairs actually used is
<= 1536 = 6*256 (vs 1608 worst case), so the host compacts the pair
space per group and phase A needs only 6 fp8-DoubleRow k-passes instead
of 7: PE drops from 59.7us to 51.2us. Falls back to KP=7 if some group
uses more pairs.

Per-core cost-model budget: PE 51.2us, DMA ~55us (7.7 table + 1.5 oh +
1.6 halo + 10.0 gram out, mostly overlapped), DVE ~49us, ACT ~42us.
"""

import sys

sys.path.insert(0, "/opt/trn_rl_repo")

import numpy as np

import concourse.bass as bass
import concourse.mybir as mybir
from concourse import bacc
from concourse.bass_utils import run_bass_kernel_spmd
from concourse.tile import TileContext

# Problem constants
NUM_LEVELS = 201
B, T, C, D = 32, 128, 8, 10000
N_CORES = 8
GB, GD = 4, 2
B_LOC = B // GB            # 8 batches per core
W = D // GD                # 5000 output cols per core
W3 = W + 3                 # incl. left halo
NJ = 40                    # d interleave: dl = NJ*p + j
NP = 128
K_TOT = C * NUM_LEVELS     # 1608
COLS = B_LOC * T           # 1024
NTP = T - 3                # 125 valid t' per batch
SHIP_P = 125               # dl = NJ*p + j < 5000 -> p <= 124

FP8 = mybir.dt.float8e4
BF16 = mybir.dt.bfloat16
F32 = mybir.dt.float32
NP_FP8 = np.dtype(mybir.dt.np(FP8))
NP_BF16 = np.dtype(mybir.dt.np(BF16))

_CACHE = {}

# j processed 0-3 (halos+head) then 36-39 (so the +2-chained late grams
# have their high-j u's early, and PE has tab-ready work while the
# deferred table stream lands); tail after the last drain (b_35) is just
# u_34, u_35 and grams 32..35.
JORDER = [0, 1, 2, 3, 37, 38, 39, 36] + list(range(4, 36))
SHIP_GROUPS = [(36, 40), (0, 4), (4, 8), (8, 12), (12, 16), (16, 20),
               (20, 24), (24, 28), (28, 30), (30, 32), (32, 34), (34, 35),
               (35, 36)]
GRP_OF = {}
for _gi, (_j0, _j1) in enumerate(SHIP_GROUPS):
    for _j in range(_j0, _j1):
        GRP_OF[_j] = _gi


def _build_program(KP):
    nc = bacc.Bacc("TRN2", target_bir_lowering=False, debug=False, num_devices=N_CORES)

    tab_p = nc.declare_dram_parameter("tab", [128, NJ, KP, 2, NP], FP8, isOutput=False)
    oh_p = nc.declare_dram_parameter("oh", [128, KP, 2, COLS], FP8, isOutput=False)
    gr_p = nc.declare_dram_parameter("gr", [NJ, SHIP_P, B_LOC * NTP], BF16, isOutput=True)

    with TileContext(nc) as tc:
        with (
            tc.tile_pool(name="const", bufs=1) as cpool,
            tc.tile_pool(name="b", bufs=6) as bpool,
            tc.tile_pool(name="u", bufs=8) as upool,
            tc.tile_pool(name="gram", bufs=3) as gpool,
            tc.tile_pool(name="ps", bufs=4, space="PSUM") as pspool,
        ):
            tab = cpool.tile([128, NJ, KP, 2, NP], FP8, tag="tab")
            oh = cpool.tile([128, KP, 2, COLS], FP8, tag="oh")

            # Loads: one-hot kp-blocks on the ACT HWDGE queue, early table
            # j-blocks on SP (the two queues' descriptor gens interleave on
            # the shared HWDGE unit, bus follows ready order); tab[36:40]
            # gated behind the oh kp2 block via a reservation copy (RAW on
            # the oh region, WAW on the tab region). The bulk table blocks
            # are emitted inside the pipeline after the halos.
            nc.sync.dma_start(out=tab[:, 0:2], in_=tab_p[:, 0:2])
            nc.scalar.dma_start(out=oh[:, 0], in_=oh_p[:, 0])
            nc.sync.dma_start(out=tab[:, 2:4], in_=tab_p[:, 2:4])
            for kp in range(1, KP):
                nc.scalar.dma_start(out=oh[:, kp], in_=oh_p[:, kp])
            nc.gpsimd.tensor_copy(out=tab[0:1, 36, 0, 0, 0:2], in_=oh[0:1, 2, 0, 0:2])
            nc.sync.dma_start(out=tab[:, 36:40], in_=tab_p[:, 36:40])

            bt = {}      # j -> b tile (0..NJ-1), NJ..NJ+2 are halos
            ut = {}      # j -> (u tile, n_partitions)
            gtiles = {}
            gdone = set()

            def mm_j(j, ps):
                for g in range(2):
                    for kp in range(KP):
                        nc.tensor.matmul(
                            ps[:, g * 512:(g + 1) * 512],
                            tab[:, j, kp],
                            oh[:, kp, :, g * 512:(g + 1) * 512],
                            start=(kp == 0),
                            stop=(kp == KP - 1),
                            perf_mode=mybir.MatmulPerfMode.DoubleRow,
                        )

            def drain_j(j, ps):
                b = bpool.tile([128, COLS + 8], BF16, tag="b", name=f"b{j}")
                nc.gpsimd.memset(b[:, COLS:], 0)
                nc.scalar.copy(out=b[:, 0:COLS], in_=ps[:])
                bt[j] = b
                if j < 3:
                    # halo: b_{NJ+j}[p] = b_j[p+1] (partition-shift DMA;
                    # on the ACT queue right after its producing drain, so
                    # its sem-wait barely parks the SEQ)
                    hl = cpool.tile([127, COLS + 2], BF16, tag=f"halo{j}", name=f"halo{j}")
                    nc.scalar.dma_start(out=hl[:], in_=b[1:128, 0:COLS + 2])
                    bt[NJ + j] = hl
                if j == 2:
                    # bulk table loads, queue-ordered behind the halos
                    for j0 in range(4, 36, 6):
                        j1 = min(j0 + 6, 36)
                        nc.scalar.dma_start(out=tab[:, j0:j1], in_=tab_p[:, j0:j1])

            def try_u(j):
                if j in ut or j not in bt or j + 1 not in bt:
                    return
                if j >= NJ and NJ - 1 not in ut:
                    # halo-u's are only needed by the late grams; emitting
                    # them early head-of-line-blocks the DVE queue on the
                    # halo DMAs
                    return
                np_ = 128 if j + 1 < NJ else 127
                u = upool.tile([128, COLS], BF16, tag="u", name=f"u{j}")
                nc.vector.tensor_mul(
                    out=u[0:np_, :],
                    in0=bt[j][0:np_, 0:COLS],
                    in1=bt[j + 1][0:np_, 1:COLS + 1],
                )
                ut[j] = (u, np_)

            def gram_ship(j):
                gi = GRP_OF[j]
                j0, j1 = SHIP_GROUPS[gi]
                if gi not in gtiles:
                    gtiles[gi] = gpool.tile(
                        [SHIP_P, (j1 - j0) * B_LOC * NTP], BF16,
                        tag=f"gram{j1 - j0}", name=f"g{gi}"
                    )
                g = gtiles[gi]
                jj = j - j0
                u0, _ = ut[j]
                u2, _ = ut[j + 2]
                in0 = u0[0:SHIP_P, 0:COLS].rearrange("p (b t) -> p b t", b=B_LOC)[:, :, 0:NTP]
                in1 = u2[0:SHIP_P, 0:COLS].rearrange("p (b t) -> p b t", b=B_LOC)[:, :, 2:NTP + 2]
                out = g[:, jj * B_LOC * NTP:(jj + 1) * B_LOC * NTP].rearrange(
                    "p (b t) -> p b t", b=B_LOC
                )
                # Mid-run grams run on the otherwise-idle Pool engine
                # (~2.1us each vs 0.59 on DVE, but it trims DVE's tail
                # backlog, which sets the finish time).
                eng = nc.gpsimd if j in POOL_G else nc.vector
                eng.tensor_mul(out=out, in0=in0, in1=in1)
                if len([x for x in range(j0, j1) if x in gdone]) == j1 - j0 - 1:
                    # Tail ships go on the (idle-by-then) ACT queue: 632ns
                    # descriptor gen vs ~1.1us on Pool.
                    eng = nc.scalar if gi >= len(SHIP_GROUPS) - 3 else nc.gpsimd
                    eng.dma_start(
                        out=gr_p[j0:j1].rearrange("j p c -> p j c"),
                        in_=g[:].rearrange("p (j c) -> p j c", j=j1 - j0),
                    )

            TAIL_GRAMS = (32, 33)
            POOL_G = frozenset()

            def emit_grams(only=None, waterfall=False):
                for j in (only if only is not None else range(NJ)):
                    if j not in gdone and j in ut and j + 2 in ut:
                        gram_ship(j)
                        gdone.add(j)
                    elif waterfall and j not in gdone:
                        break

            def advance():
                # Interleave: after each new u, immediately emit the
                # tail-critical grams it unlocks IN SHIP-GROUP ORDER
                # (waterfall: don't jump ahead of a not-yet-ready gram, so
                # each ship group completes as early as possible).
                emit_grams()
                for j in range(NJ + 2):
                    before = j in ut
                    try_u(j)
                    if not before and j in ut:
                        emit_grams(TAIL_GRAMS, waterfall=True)
                emit_grams()

            # Head: kp-major over j0-1 then j2-3 so PE starts on the first
            # one-hot kp-block and the first drains land early.
            HEAD = JORDER[:4]
            head_ps = {j: pspool.tile([128, COLS], F32, tag="ps", name=f"psh{j}") for j in HEAD}
            for jpair in (HEAD[0:2], HEAD[2:4]):
                for kp in range(KP):
                    for j in jpair:
                        for g in range(2):
                            nc.tensor.matmul(
                                head_ps[j][:, g * 512:(g + 1) * 512],
                                tab[:, j, kp],
                                oh[:, kp, :, g * 512:(g + 1) * 512],
                                start=(kp == 0),
                                stop=(kp == KP - 1),
                                perf_mode=mybir.MatmulPerfMode.DoubleRow,
                            )
                for j in jpair:
                    drain_j(j, head_ps[j])
                    advance()

            for j in JORDER[4:]:
                ps = pspool.tile([128, COLS], F32, tag="ps", name=f"ps{j}")
                mm_j(j, ps)
                drain_j(j, ps)
                advance()
            advance()
            assert len(gdone) == NJ, f"grams stuck: {sorted(set(range(NJ)) - gdone)}"

    nc.finalize()
    return nc


def _host_prep(x, level_hv, channel_hv):
    # Bit-exact replication of the jax fp32 quantization
    x = np.asarray(x, dtype=np.float32)
    t1 = x + np.float32(100.0)
    t2 = t1 / np.float32(200.0)
    t3 = t2 * np.float32(200.0)
    idx = np.clip(np.rint(t3), 0, NUM_LEVELS - 1).astype(np.int32)  # [B,T,C]

    fp8_one = np.float32(1.0).astype(NP_FP8)
    fp8_mone = np.float32(-1.0).astype(NP_FP8)

    # folded +-1 table [K_TOT, D] fp8, pair id k = c*201 + level
    prod = (channel_hv[:, None, :] * level_hv[None, :, :]).reshape(K_TOT, D)
    tabf = np.where(prod > 0, fp8_one, fp8_mone)

    # Per-group pair compaction: with 8 batches/group the used-pair count
    # is <= 1536 (checked dynamically), enabling 6 k-passes.
    cc = np.arange(C)[None, None, :]
    kk_all = cc * NUM_LEVELS + idx                      # [B, T, C]
    groups = []
    for gb in range(GB):
        kk = kk_all[gb * B_LOC:(gb + 1) * B_LOC]        # [B_LOC, T, C]
        used = np.unique(kk)
        groups.append((kk, used))
    n_max = max(len(u) for _, u in groups)
    KP = 6 if n_max <= 6 * 256 else 7
    K_PAD = KP * 256

    ohs, tabs = [], []
    for gb in range(GB):
        kk, used = groups[gb]
        slot = np.full(K_TOT, 0, np.int32)
        slot[used] = np.arange(len(used))
        # one-hot on compacted slots: [K_PAD, COLS]
        oh = np.zeros((K_PAD, COLS), dtype=NP_FP8)
        bb, tt, ccg = np.meshgrid(np.arange(B_LOC), np.arange(T), np.arange(C), indexing="ij")
        oh[slot[kk].ravel(), (bb * T + tt).ravel()] = fp8_one
        ohs.append(np.ascontiguousarray(
            oh.reshape(KP, 2, 128, COLS).transpose(2, 0, 1, 3)))
        # compacted table rows for this group, per d-window
        row = np.zeros((K_PAD, D), dtype=NP_FP8)
        row[:len(used)] = tabf[used]
        per_gd = []
        for gd in range(GD):
            dls = NJ * np.arange(NP)[None, :] + np.arange(NJ)[:, None]  # [NJ, NP]
            cols = (gd * W - 3 + dls) % D
            tcore = np.zeros((K_PAD, NJ, NP), dtype=NP_FP8)
            valid = dls < W3
            tcore[:, valid] = row[:, cols[valid]]
            per_gd.append(np.ascontiguousarray(
                tcore.reshape(KP, 2, 128, NJ, NP).transpose(2, 3, 0, 1, 4)))
        tabs.append(per_gd)
    return KP, tabs, ohs


def kernel(x, level_hv, channel_hv, centroid):
    KP, tabs, ohs = _host_prep(x, level_hv, channel_hv)
    if ("nc", KP) not in _CACHE:
        _CACHE[("nc", KP)] = _build_program(KP)
    nc = _CACHE[("nc", KP)]
    _CACHE["nc"] = nc

    in_maps = []
    for core in range(N_CORES):
        gb, gd = divmod(core, GD)
        in_maps.append({"tab": tabs[gb][gd], "oh": ohs[gb]})

    res = run_bass_kernel_spmd(nc, in_maps, list(range(N_CORES)))
    _CACHE["last_results"] = res

    sample = np.zeros((B, D), dtype=np.float32)
    dl = NJ * np.arange(SHIP_P)[None, :] + np.arange(NJ)[:, None]  # [NJ, P]
    ok = dl < W
    for core in range(N_CORES):
        gb, gd = divmod(core, GD)
        gr = res.results[core]["gr"]  # [NJ, SHIP_P, B_LOC*NTP] bf16
        g32 = (np.ascontiguousarray(gr).view(np.uint16).astype(np.uint32) << 16).view(np.float32)
        s = g32.reshape(NJ, SHIP_P, B_LOC, NTP).sum(axis=-1)  # [NJ, P, B_LOC]
        for b2 in range(B_LOC):
            row = np.zeros(W, dtype=np.float32)
            row[dl[ok]] = s[:, :, b2][ok]
            sample[gb * B_LOC + b2, gd * W:(gd + 1) * W] = row
    sign = np.where(sample > 0, np.float32(1.0), np.float32(-1.0))
    return (sign @ np.asarray(centroid, dtype=np.float32).T).astype(np.float32)
